# revision 11
# baseline (speedup 1.0000x reference)
"""VQ codebook reconstruction kernel for Trainium2 (8 NeuronCores, SPMD).

Reference computation (per pixel feature vector f in R^C):
    weights = (codebook @ f) / ||codebook_rows||^2      # [N]
    recon   = codebook.T @ weights                      # [C]

This collapses to a single fixed matrix applied per pixel:
    recon = M @ f,   M = codebook.T @ diag(1/||c_n||^2) @ codebook   # [C, C]

M is tiny ([256,256]) and is formed on the host in float64; the device
kernel applies M to all B*H*W = 131072 pixel vectors, sharded
data-parallel over (B, H) across 8 cores.

The kernel is DMA-bound: 16.78 MB fp32 read + 8.39 MB fp16 write per
core against a ~420 GB/s 16-engine SDMA pool (~200 GB/s per ring) =>
~61 us of pure DMA. Schedule highlights:
  * M is prepended to the feature shard on the host, so the first sync
    HWDGE DMA delivers M + slab 0 together under ONE completion
    semaphore -> first matmul ~12 us in.
  * Reads go ONLY on the sync/scalar HWDGE rings: the gpsimd SWDGE
    ring's completion semaphores lag the data by ~10 us, which stalls
    any compute waiting on them. Writes (nothing waits on their
    completion) use gpsimd, joining from ~16 us.
  * Only 8 DMA completion-semaphore lanes exist; issuing DMA #N+8
    stalls the issuing ENGINE until DMA #N completes. DMA emissions
    are therefore interleaved in natural temporal order (reads 0-4 up
    front, then per slab: write j, read j+5) so every lane-reuse wait
    targets an already-finished DMA, and all late-read issues sit on
    the otherwise-idle sync engine - never on the cast engines.
  * Slab sizes taper (512..2048..512): compute starts early and the
    final read->matmul->cast->write chain is short.
  * Matmuls keep the kb-adjacent per-region order (PE runs it at mid
    p-state, 475 ns per 512-col matmul; LDWEIGHTS cannot be hoisted
    for f32r).
  * PSUM->SBUF fp32->fp16 casts are split: vector(DVE) takes row-half
    0, scalar(ACT, fast PSUM port) takes row-half 1, 1024 cols per
    instruction to amortize the cayman SBUF-op errata bubble.
"""

import numpy as np

B, C, H, W = 4, 256, 128, 256
N_CORES = 8
SPLIT_H = 2            # 8 shards = B(4) x H-halves(2)
SH = H // SPLIT_H      # 64 rows of H per shard
P_SHARD = SH * W       # 16384 pixels per core
P_EXT = 256 + P_SHARD  # M's 256 columns prepended

SLABS = [512, 512, 1024, 2048, 2048, 2048, 2048, 2048, 2048, 1024, 512, 512]
assert sum(SLABS) == P_SHARD

_NC_CACHE = {}


def _build_nc():
    if "nc" in _NC_CACHE:
        return _NC_CACHE["nc"]

    import concourse.bass as bass
    import concourse.tile as tile
    from concourse import bacc, mybir

    f32 = mybir.dt.float32
    f16 = mybir.dt.float16
    f32r = mybir.dt.float32r

    nc = bacc.Bacc()
    feat = nc.dram_tensor("feat", [C, P_EXT], f32r, kind="ExternalInput")
    # fp16 output halves write traffic; host upcasts to fp32 (exact).
    out = nc.dram_tensor("out", [C, P_SHARD], f16, kind="ExternalOutput")

    feat3 = feat.rearrange("(a k) n -> k a n", a=2)
    out3 = out.rearrange("(a p) n -> p a n", a=2)

    # Which engine/ring issues each read (0 = the fused M+slab0 DMA).
    # Strict alternation keeps both HWDGE rings' delivery order matched
    # to the consumption order and their byte loads balanced.
    READ_ENG = {0: "sync", 1: "scalar", 2: "sync", 3: "scalar",
                4: "sync", 5: "scalar", 6: "sync", 7: "scalar",
                8: "sync", 9: "scalar", 10: "sync", 11: "scalar"}
    # When each read is EMITTED: reads 0-4 up front; the rest are
    # pushed down the program so their sem-lane-reuse waits target
    # long-finished DMAs and never stall the cast engines.
    READ_EMIT = {5: 0, 6: 1, 7: 2, 8: 3, 9: 6, 10: 5, 11: 8}
    # Bulk writes on gpsimd; late writes fan out to the HWDGE rings.
    WRITE_ENG = {7: "scalar", 8: "sync", 9: "scalar", 10: "sync",
                 11: "scalar"}

    with tile.TileContext(nc) as tc:
        with (
            tc.tile_pool(name="rhs", bufs=1) as rhs_pool,
            tc.tile_pool(name="opool", bufs=4) as opool,
            tc.tile_pool(name="psum", bufs=2, space="PSUM") as psum_pool,
        ):
            eng = lambda name: getattr(nc, name)
            offs = [256]
            for S in SLABS:
                offs.append(offs[-1] + S)

            rts = [None] * 12

            def emit_read(j):
                lo = offs[j] - (256 if j == 0 else 0)
                S = offs[j + 1] - lo
                rt = rhs_pool.tile([128, 2, S], f32r, tag=f"r{j}",
                                   name=f"rt{j}")
                eng(READ_ENG[j]).dma_start(rt[:], feat3[:, :, lo:lo + S])
                rts[j] = rt

            # Reads 0-4 up front (5 DMAs, within the 8 sem lanes).
            for j in range(5):
                emit_read(j)

            mt = rts[0][:, :, 0:256]   # the fused M block

            for j, S in enumerate(SLABS):
                rt = rts[j] if j else rts[0][:, :, 256:]
                ot = opool.tile([128, 2, S], f16, tag="o", name=f"ot{j}")
                NG = (S + 1023) // 1024           # 1024-col cast groups
                for g in range(NG):
                    GW = min(1024, S - g * 1024)
                    ps = [
                        psum_pool.tile([128, 1024], f32, tag=f"ps{mb}",
                                       name=f"ps{j}_{g}_{mb}")
                        for mb in range(2)
                    ]
                    for n2 in range(GW // 512):
                        n = g * 2 + n2            # 512-col region in slab
                        for mb in range(2):
                            for kb in range(2):
                                nc.tensor.matmul(
                                    ps[mb][:, n2 * 512:(n2 + 1) * 512],
                                    mt[:, kb, mb * 128:(mb + 1) * 128],
                                    rt[:, kb, n * 512:(n + 1) * 512],
                                    start=(kb == 0),
                                    stop=(kb == 1),
                                )
                    lo, hi = g * 1024, g * 1024 + GW
                    nc.vector.tensor_copy(ot[:, 0, lo:hi], ps[0][:, :GW])
                    nc.scalar.copy(ot[:, 1, lo:hi], ps[1][:, :GW])
                olo = offs[j] - 256
                eng(WRITE_ENG.get(j, "gpsimd")).dma_start(
                    out3[:, :, olo:olo + S], ot[:]
                )
                for rj, at in READ_EMIT.items():
                    if at == j:
                        emit_read(rj)

    nc.compile()
    _NC_CACHE["nc"] = nc
    return nc


def _host_prep(feature, codebook):
    cb = codebook.astype(np.float64)
    norm = np.sum(cb * cb, axis=1)
    m = ((cb / norm[:, None]).T @ cb).astype(np.float32)

    in_maps = []
    for i in range(N_CORES):
        b, hs = i // SPLIT_H, (i % SPLIT_H) * SH
        shard = feature[b, :, hs:hs + SH, :].reshape(C, P_SHARD)
        ext = np.concatenate([m, shard], axis=1)  # [C, 256 + P_SHARD]
        in_maps.append({"feat": np.ascontiguousarray(ext)})
    return in_maps


def _gather(results):
    out = np.empty((B, C, H, W), dtype=np.float32)
    for i in range(N_CORES):
        b, hs = i // SPLIT_H, (i % SPLIT_H) * SH
        out[b, :, hs:hs + SH, :] = results[i]["out"].reshape(C, SH, W).astype(np.float32)
    return out


def run(feature, codebook, **spmd_kwargs):
    from concourse.bass_utils import run_bass_kernel_spmd

    nc = _build_nc()
    in_maps = _host_prep(np.asarray(feature), np.asarray(codebook))
    res = run_bass_kernel_spmd(nc, in_maps, list(range(N_CORES)), **spmd_kwargs)
    return _gather(res.results), res


def kernel(feature, codebook):
    out, _ = run(feature, codebook)
    return out
